# revision 7
# baseline (speedup 1.0000x reference)
"""Trainium2 Bass kernel for per-image masked-softmax entropy (EntropyLoss).

Math (per (n, c) segment, over the HW=512*512 elements x of heatmap[n, c]):
    mask  = x > 0
    softmax over the masked elements, entropy in bits, summed over c and
    divided by the total positive count of image n.

The entropy of a masked softmax is invariant to the stabilizing shift m, so
we may use m = 0 (randn inputs keep exp(x) <= ~e^6, no overflow):
    S_c   = sum_{x>0} exp(x)
    U_c   = sum_{x>0} x * exp(x)
    ent_c = (log S_c - U_c / S_c) / ln2          [bits]
    out_n = sum_c ent_c / sum_c count_c

Device work per segment tile [128, 2048] (bf16 x, cast during DMA):
    r  = relu(x)                  (DVE tensor_scalar, 4x bf16)
    a  = exp(r)                   (ACT)
    w  = a * r                    (DVE tensor_tensor, 2x bf16)
    mk = x > 0 with accum_out     (DVE tensor_scalar 4x; accum gives the
                                   per-partition positive count directly)
    S'_c, U_c                     (PE: one-hot stationary weights route each
                                   segment's column sums of a and w into PSUM
                                   row c of [20, 512] accumulators; host sums
                                   the 512 columns)
S_c is recovered on the host as S'_c - (HW - count_c) since exp(0) = 1 for
every non-positive element. Final log/divide runs on host in float64.
"""

import os

import numpy as np

N, C, H, W = 8, 20, 512, 512
HW = H * W
P = 128
F = HW // P  # 2048
NCORES = 8
LN2 = 0.6931471805599453

DATA_BUFS = int(os.environ.get("ENTROPY_DATA_BUFS", "8"))
WARM_MM = int(os.environ.get("ENTROPY_WARM_MM", "12"))
WARM_VE = int(os.environ.get("ENTROPY_WARM_VE", "4"))

# Work items: (segment, dram col offset, width). Narrow items at both ends
# shorten pipeline ramp and drain; interior items are full segments.
ITEMS = (
    [(0, 0, 512), (0, 512, 512), (0, 1024, 1024)]
    + [(c, 0, F) for c in range(1, C - 1)]
    + [(C - 1, 0, 1024), (C - 1, 1024, 512), (C - 1, 1536, 512)]
)
NITEMS = len(ITEMS)

_CACHE = {}


def _build_program():
    import concourse.bacc as bacc
    import concourse.mybir as mybir
    import concourse.tile as tile

    dt = mybir.dt
    Alu = mybir.AluOpType
    Act = mybir.ActivationFunctionType

    nc = bacc.Bacc(None, target_bir_lowering=False, debug=False)

    x_dram = nc.dram_tensor("x", [C, P, F], dt.float32, kind="ExternalInput")
    su_dram = nc.dram_tensor("su_out", [C, 2], dt.float32, kind="ExternalOutput")
    n_dram = nc.dram_tensor("n_out", [P, NITEMS], dt.float32, kind="ExternalOutput")

    with tile.TileContext(nc) as tc:
        with (
            tc.tile_pool(name="const", bufs=1) as constp,
            tc.tile_pool(name="res", bufs=1) as resp,
            tc.tile_pool(name="data", bufs=DATA_BUFS) as datap,
            tc.tile_pool(name="psum", bufs=1, space="PSUM") as psump,
        ):
            # Sliding-window one-hot weights: oh[:, 20 - c : 40 - c] is a
            # [128, 20] matrix whose only nonzero column (all ones) is c.
            oh = constp.tile([P, 2 * C], dt.bfloat16)
            nc.gpsimd.memset(oh[:], 0.0)
            nc.gpsimd.memset(oh[:, C : C + 1], 1.0)

            n_res = resp.tile([P, NITEMS], dt.float32)
            su_red = resp.tile([C, 2], dt.float32)

            u_psum = psump.tile([C, 512], dt.float32)
            s_psum = psump.tile([C, 512], dt.float32)

            # Engine warmup during the DMA fill phase: dummy matmuls keep the
            # PE busy so HAM upclocks it before real work lands; same for the
            # vector/scalar engines.
            warm = constp.tile([P, 512], dt.bfloat16)
            nc.vector.memset(warm[:], 0.0)
            if WARM_MM:
                w_psum = psump.tile([C, 512], dt.float32)
                for i in range(WARM_MM):
                    nc.tensor.matmul(
                        w_psum[:], oh[:, 0:C], warm[:],
                        start=(i == 0), stop=(i == WARM_MM - 1),
                    )
            if WARM_VE:
                warm2 = constp.tile([P, 512], dt.bfloat16)
                warm3 = constp.tile([P, 512], dt.bfloat16)
                for i in range(WARM_VE):
                    nc.vector.tensor_scalar(warm2[:], warm[:], 0.0, None, Alu.max)
                    nc.scalar.activation(warm3[:], warm[:], Act.Exp)

            mm_first = [True, True]  # [s stream, u stream]

            def emit_mms(psum_i, lhsT, rhs, width, last):
                first = mm_first[psum_i]
                mm_first[psum_i] = False
                tgt = u_psum if psum_i else s_psum
                nj = width // 512
                for j in range(nj):
                    nc.tensor.matmul(
                        tgt[:],
                        lhsT,
                        rhs[:, j * 512 : (j + 1) * 512],
                        start=(first and j == 0),
                        stop=(last and j == nj - 1),
                    )

            pending = None  # (c, width, r_t, a_t) awaiting mult + PE
            for idx, (c, lo, width) in enumerate(ITEMS):
                x_t = datap.tile([P, width], dt.bfloat16, tag="x")
                # SWDGE DMA casts fp32 -> bf16 on the fly.
                nc.gpsimd.dma_start(x_t[:], x_dram[c, :, lo : lo + width])

                r_t = datap.tile([P, width], dt.bfloat16, tag="r")
                a_t = datap.tile([P, width], dt.bfloat16, tag="a")
                nc.vector.tensor_scalar(r_t[:], x_t[:], 0.0, None, Alu.max)
                nc.scalar.activation(a_t[:], r_t[:], Act.Exp)

                # One item of software pipelining: the previous item's
                # mult/matmuls run while this item's exp is in flight.
                if pending is not None:
                    pc, pw, pr, pa = pending
                    w_t = datap.tile([P, pw], dt.bfloat16, tag="w")
                    nc.vector.tensor_tensor(w_t[:], pa[:], pr[:], Alu.mult)
                    lhsT = oh[:, C - pc : 2 * C - pc]
                    emit_mms(0, lhsT, pa, pw, False)
                    emit_mms(1, lhsT, w_t, pw, False)

                mk_t = datap.tile([P, width], dt.bfloat16, tag="mk")
                nc.vector.tensor_scalar(
                    mk_t[:], x_t[:], 0.0, 0.0, Alu.is_gt, Alu.add,
                    accum_out=n_res[:, idx : idx + 1],
                )
                pending = (c, width, r_t, a_t)

            pc, pw, pr, pa = pending
            w_t = datap.tile([P, pw], dt.bfloat16, tag="w")
            nc.vector.tensor_tensor(w_t[:], pa[:], pr[:], Alu.mult)
            lhsT = oh[:, C - pc : 2 * C - pc]
            emit_mms(0, lhsT, pa, pw, True)
            emit_mms(1, lhsT, w_t, pw, True)

            nc.vector.tensor_reduce(
                su_red[:, 0:1], s_psum[:], mybir.AxisListType.X, Alu.add
            )
            nc.vector.tensor_reduce(
                su_red[:, 1:2], u_psum[:], mybir.AxisListType.X, Alu.add
            )
            nc.sync.dma_start(su_dram[:], su_red[:])
            nc.sync.dma_start(n_dram[:], n_res[:])

    nc.compile()
    return nc


def _get_program():
    if "nc" not in _CACHE:
        _CACHE["nc"] = _build_program()
    return _CACHE["nc"]


def _run(heatmap: np.ndarray, trace: bool = False):
    from concourse.bass_utils import run_bass_kernel_spmd

    nc = _get_program()
    in_maps = [
        {"x": np.ascontiguousarray(heatmap[i].reshape(C, P, F), dtype=np.float32)}
        for i in range(NCORES)
    ]
    return run_bass_kernel_spmd(nc, in_maps, list(range(NCORES)), trace=trace)


def _finalize(results) -> np.ndarray:
    """Host epilogue: a few hundred scalars per core -> entropy[n]."""
    out = np.zeros(N, dtype=np.float64)
    for n in range(NCORES):
        r = results[n]
        s_prime = r["su_out"].astype(np.float64)[:, 0]          # [C]
        u = r["su_out"].astype(np.float64)[:, 1]                # [C]
        n_it = r["n_out"].astype(np.float64).sum(axis=0)        # [NITEMS]
        cnt = np.zeros(C, dtype=np.float64)
        for idx, (c, _, _) in enumerate(ITEMS):
            cnt[c] += n_it[idx]
        s = s_prime - (HW - cnt)                                # masked sum exp
        ent = np.zeros(C, dtype=np.float64)
        ok = s > 0
        ent[ok] = (np.log(s[ok]) - u[ok] / s[ok]) / LN2
        out[n] = ent.sum() / cnt.sum()
    return out.astype(np.float32)


def kernel(heatmap: np.ndarray) -> np.ndarray:
    heatmap = np.asarray(heatmap, dtype=np.float32)
    assert heatmap.shape == (N, C, H, W), heatmap.shape
    res = _run(heatmap, trace=False)
    return _finalize(res.results)


# revision 13
# speedup vs baseline: 1.1756x; 1.1756x over previous
"""Trainium2 Bass kernel for per-image masked-softmax entropy (EntropyLoss).

Math (per (n, c) segment, over the HW=512*512 elements x of heatmap[n, c]):
    mask  = x > 0
    softmax over the masked elements, entropy in bits, summed over c and
    divided by the total positive count of image n.

The entropy of a masked softmax is invariant to the stabilizing shift m, so
we may use m = 0 (randn inputs keep exp(x) <= ~e^6, no overflow):
    S_c   = sum_{x>0} exp(x)
    U_c   = sum_{x>0} x * exp(x)
    ent_c = (log S_c - U_c / S_c) / ln2          [bits]
    out_n = sum_c ent_c / sum_c count_c

Device work per pair-of-segments tile [128, 4096] (bf16 x, cast during DMA):
    r  = relu(x)                 (DVE tensor_scalar, 4x bf16, pair-wide)
    a  = exp(r)                  (ACT per segment, fused accum -> S'_c)
    w  = a * r                   (DVE tensor_tensor, 2x bf16, pair-wide)
    mk = x > 0                   (DVE tensor_scalar, 4x bf16, pair-wide)
    U_c, count_c                 (PE: one-hot stationary weights route each
                                  segment's column sums of w / mk into PSUM
                                  row c of [20, 512] accumulators; ACT
                                  Copy+accum folds 512 -> 1 per row)
S_c is recovered on the host as S'_c - (HW - count_c) since exp(0) = 1 for
every non-positive element. Final log/divide runs on host in float64.
"""

import os

import numpy as np

N, C, H, W = 8, 20, 512, 512
HW = H * W
P = 128
F = HW // P  # 2048
NCORES = 8
LN2 = 0.6931471805599453

DATA_BUFS = int(os.environ.get("ENTROPY_DATA_BUFS", "4"))
WARM_MM = int(os.environ.get("ENTROPY_WARM_MM", "16"))
WARM_VE = int(os.environ.get("ENTROPY_WARM_VE", "4"))

# DMA items: (first segment, [widths of sub-items]). A sub-item never spans
# a segment boundary; sub-item exp gets its own accumulator column.
# First and last pairs are split narrow for fast pipeline ramp/drain.
PAIRS = (
    [(0, [1024, 1024, 1024, 1024])]
    + [(c, [F, F]) for c in range(2, C - 2, 2)]
    + [(C - 2, [1024, 1024, 1024, 512, 512])]
)
# exp item list: (segment, accum column) in order
EXP_ITEMS = []
for _c0, _ws in PAIRS:
    _off = 0
    for _w in _ws:
        EXP_ITEMS.append((_c0 + _off // F, _w))
        _off += _w
NEXP = len(EXP_ITEMS)

_CACHE = {}


def _build_program():
    import concourse.bacc as bacc
    import concourse.mybir as mybir
    import concourse.tile as tile

    dt = mybir.dt
    Alu = mybir.AluOpType
    Act = mybir.ActivationFunctionType

    nc = bacc.Bacc(None, target_bir_lowering=False, debug=False)

    x_dram = nc.dram_tensor("x", [C, P, F], dt.float32, kind="ExternalInput")
    s_dram = nc.dram_tensor("s_out", [P, NEXP], dt.float32, kind="ExternalOutput")
    un_dram = nc.dram_tensor("un_out", [C, 4], dt.float32, kind="ExternalOutput")

    with tile.TileContext(nc) as tc:
        with (
            tc.tile_pool(name="const", bufs=1) as constp,
            tc.tile_pool(name="res", bufs=1) as resp,
            tc.tile_pool(name="data", bufs=DATA_BUFS) as datap,
            tc.tile_pool(name="psum", bufs=1, space="PSUM") as psump,
        ):
            # Sliding-window one-hot weights: oh[:, 20 - c : 40 - c] is a
            # [128, 20] matrix whose only nonzero column (all ones) is c.
            oh = constp.tile([P, 2 * C], dt.bfloat16)
            nc.gpsimd.memset(oh[:], 0.0)
            nc.gpsimd.memset(oh[:, C : C + 1], 1.0)

            s_res = resp.tile([P, NEXP], dt.float32)
            un_red = resp.tile([C, 4], dt.float32)
            red_scratch = resp.tile([C, 512], dt.float32)

            # Two PSUM bank sets: set 0 accumulates the first half of the
            # pairs and its 512->1 fold runs mid-kernel, hiding its latency.
            u_psum = [
                psump.tile([C, 512], dt.float32, name=f"u_psum{i}") for i in range(2)
            ]
            c_psum = [
                psump.tile([C, 512], dt.float32, name=f"c_psum{i}") for i in range(2)
            ]

            # Engine warmup during the DMA fill phase (HAM clock ramp).
            warm = constp.tile([P, 512], dt.bfloat16)
            nc.vector.memset(warm[:], 0.0)
            if WARM_MM:
                w_psum = psump.tile([C, 512], dt.float32)
                for i in range(WARM_MM):
                    nc.tensor.matmul(
                        w_psum[:], oh[:, 0:C], warm[:],
                        start=(i == 0), stop=(i == WARM_MM - 1),
                    )
            if WARM_VE:
                warm2 = constp.tile([P, 512], dt.bfloat16)
                warm3 = constp.tile([P, 512], dt.bfloat16)
                for i in range(WARM_VE):
                    nc.vector.tensor_scalar(warm2[:], warm[:], 0.0, None, Alu.max)
                    nc.scalar.activation(warm3[:], warm[:], Act.Exp)

            npairs = len(PAIRS)
            half = npairs // 2  # pairs [0, half) -> bank set 0
            mm_state = {(0, 0): True, (0, 1): True, (1, 0): True, (1, 1): True}

            def emit_mms(stream, bank, rhs, c0, width, last=False):
                """Route 512-col chunks of a pair-wide rhs into the PSUM row
                of the segment each chunk belongs to (c0 + col // F)."""
                tgt = (u_psum if stream == 0 else c_psum)[bank]
                nj = width // 512
                for j in range(nj):
                    c = c0 + (j * 512) // F
                    first = mm_state[(stream, bank)]
                    mm_state[(stream, bank)] = False
                    nc.tensor.matmul(
                        tgt[:],
                        oh[:, C - c : 2 * C - c],
                        rhs[:, j * 512 : (j + 1) * 512],
                        start=first,
                        stop=last and j == nj - 1,
                    )

            def fold(bank):
                """512->1 fold of bank's PSUM rows on the ACT engine."""
                nc.scalar.activation(
                    red_scratch[:], u_psum[bank][:], Act.Copy,
                    accum_out=un_red[:, bank : bank + 1],
                )
                nc.scalar.activation(
                    red_scratch[:], c_psum[bank][:], Act.Copy,
                    accum_out=un_red[:, 2 + bank : 2 + bank + 1],
                )

            exp_col = 0
            pending = None  # (pair_idx, c0, width, r_t, a_t)
            for pi, (c0, widths) in enumerate(PAIRS):
                pw = sum(widths)
                bank = 0 if pi < half else 1
                x_t = datap.tile([P, pw], dt.bfloat16, tag="x")
                # SWDGE DMA casts fp32 -> bf16 on the fly; one DMA per pair.
                nc.gpsimd.dma_start(x_t[:, 0:F], x_dram[c0, :, :])
                nc.gpsimd.dma_start(x_t[:, F : 2 * F], x_dram[c0 + 1, :, :])

                r_t = datap.tile([P, pw], dt.bfloat16, tag="r")
                a_t = datap.tile([P, pw], dt.bfloat16, tag="a")
                mk_t = datap.tile([P, pw], dt.bfloat16, tag="mk")

                nc.vector.tensor_scalar(r_t[:], x_t[:], 0.0, None, Alu.max)
                off = 0
                for w in widths:
                    nc.scalar.activation(
                        a_t[:, off : off + w], r_t[:, off : off + w], Act.Exp,
                        accum_out=s_res[:, exp_col : exp_col + 1],
                    )
                    exp_col += 1
                    off += w
                nc.vector.tensor_scalar(mk_t[:], x_t[:], 0.0, None, Alu.is_gt)

                if pending is not None:
                    ppi, pc0, ppw, pr, pa, pmk = pending
                    pbank = 0 if ppi < half else 1
                    blast = ppi == half - 1  # last pair of bank set 0
                    w_t = datap.tile([P, ppw], dt.bfloat16, tag="w")
                    nc.vector.tensor_tensor(w_t[:], pa[:], pr[:], Alu.mult)
                    emit_mms(1, pbank, pmk, pc0, ppw, last=blast)
                    emit_mms(0, pbank, w_t, pc0, ppw, last=blast)
                    if blast:
                        fold(0)

                pending = (pi, c0, pw, r_t, a_t, mk_t)

            ppi, pc0, ppw, pr, pa, pmk = pending
            w_t = datap.tile([P, ppw], dt.bfloat16, tag="w")
            nc.vector.tensor_tensor(w_t[:], pa[:], pr[:], Alu.mult)
            emit_mms(1, 1, pmk, pc0, ppw, last=True)
            emit_mms(0, 1, w_t, pc0, ppw, last=True)
            fold(1)
            nc.sync.dma_start(s_dram[:], s_res[:])
            nc.sync.dma_start(un_dram[:], un_red[:])

    nc.compile()
    return nc


def _get_program():
    if "nc" not in _CACHE:
        _CACHE["nc"] = _build_program()
    return _CACHE["nc"]


def _run(heatmap: np.ndarray, trace: bool = False):
    from concourse.bass_utils import run_bass_kernel_spmd

    nc = _get_program()
    in_maps = [
        {"x": np.ascontiguousarray(heatmap[i].reshape(C, P, F), dtype=np.float32)}
        for i in range(NCORES)
    ]
    return run_bass_kernel_spmd(nc, in_maps, list(range(NCORES)), trace=trace)


def _finalize(results) -> np.ndarray:
    """Host epilogue: a few hundred scalars per core -> entropy[n]."""
    out = np.zeros(N, dtype=np.float64)
    for n in range(NCORES):
        r = results[n]
        s_it = r["s_out"].astype(np.float64).sum(axis=0)        # [NEXP]
        s_prime = np.zeros(C, dtype=np.float64)
        for idx, (c, _) in enumerate(EXP_ITEMS):
            s_prime[c] += s_it[idx]
        un = r["un_out"].astype(np.float64)                     # [C, 4]
        u = un[:, 0] + un[:, 1]
        cnt = un[:, 2] + un[:, 3]
        s = s_prime - (HW - cnt)                                # masked sum exp
        ent = np.zeros(C, dtype=np.float64)
        ok = s > 0
        ent[ok] = (np.log(s[ok]) - u[ok] / s[ok]) / LN2
        out[n] = ent.sum() / cnt.sum()
    return out.astype(np.float32)


def kernel(heatmap: np.ndarray) -> np.ndarray:
    heatmap = np.asarray(heatmap, dtype=np.float32)
    assert heatmap.shape == (N, C, H, W), heatmap.shape
    res = _run(heatmap, trace=False)
    return _finalize(res.results)


# revision 14
# speedup vs baseline: 1.3651x; 1.1612x over previous
"""Trainium2 Bass kernel for per-image masked-softmax entropy (EntropyLoss).

Math (per (n, c) segment, over the HW=512*512 elements x of heatmap[n, c]):
    mask  = x > 0
    softmax over the masked elements, entropy in bits, summed over c and
    divided by the total positive count of image n.

The entropy of a masked softmax is invariant to the stabilizing shift m, so
we may use m = 0 (randn inputs keep exp(x) <= ~e^6, no overflow):
    S_c   = sum_{x>0} exp(x)
    U_c   = sum_{x>0} x * exp(x)
    ent_c = (log S_c - U_c / S_c) / ln2          [bits]
    out_n = sum_c ent_c / sum_c count_c

Device work per segment tile [128, 2048] (bf16 x, cast during DMA):
    r  = relu(x)                 (DVE tensor_scalar, 4x bf16)
    a  = exp(r)                  (ACT, fused accum -> S'_c partial)
    mk = x > 0                   (DVE tensor_scalar, 4x bf16)
    w  = a * r                   (DVE tensor_tensor, 2x bf16; emitted one
                                  item late so the DVE never stalls on ACT)
    U_c, count_c                 (PE: one-hot stationary weights route each
                                  segment's column sums of w / mk into PSUM
                                  row c of [20, 512] accumulators, two bank
                                  sets; ACT Copy+accum folds 512 -> 1, the
                                  first set mid-kernel)
S_c is recovered on the host as S'_c - (HW - count_c) since exp(0) = 1 for
every non-positive element. Final log/divide runs on host in float64.
"""

import os

import numpy as np

N, C, H, W = 8, 20, 512, 512
HW = H * W
P = 128
F = HW // P  # 2048
NCORES = 8
LN2 = 0.6931471805599453

DATA_BUFS = int(os.environ.get("ENTROPY_DATA_BUFS", "8"))
WARM_MM = int(os.environ.get("ENTROPY_WARM_MM", "16"))
WARM_VE = int(os.environ.get("ENTROPY_WARM_VE", "4"))

# Work items: (segment, dram col offset, width). Narrow items at the ends
# shorten pipeline ramp and drain.
ITEMS = (
    [(0, 0, 1024), (0, 1024, 1024), (1, 0, 1024), (1, 1024, 1024)]
    + [(c, 0, F) for c in range(2, C - 2)]
    + [(C - 2, 0, 1024), (C - 2, 1024, 1024)]
    + [(C - 1, 0, 1024), (C - 1, 1024, 512), (C - 1, 1536, 512)]
)
NITEMS = len(ITEMS)
# Items whose segment < 10 accumulate in PSUM bank set 0 (folded mid-kernel).
BANK0_LAST = max(i for i, (c, _, _) in enumerate(ITEMS) if c < C // 2)

_CACHE = {}


def _build_program():
    import concourse.bacc as bacc
    import concourse.mybir as mybir
    import concourse.tile as tile

    dt = mybir.dt
    Alu = mybir.AluOpType
    Act = mybir.ActivationFunctionType

    nc = bacc.Bacc(None, target_bir_lowering=False, debug=False)

    x_dram = nc.dram_tensor("x", [C, P, F], dt.float32, kind="ExternalInput")
    s_dram = nc.dram_tensor("s_out", [P, NITEMS], dt.float32, kind="ExternalOutput")
    un_dram = nc.dram_tensor("un_out", [C, 4], dt.float32, kind="ExternalOutput")

    with tile.TileContext(nc) as tc:
        with (
            tc.tile_pool(name="const", bufs=1) as constp,
            tc.tile_pool(name="res", bufs=1) as resp,
            tc.tile_pool(name="data", bufs=DATA_BUFS) as datap,
            tc.tile_pool(name="psum", bufs=1, space="PSUM") as psump,
        ):
            # Sliding-window one-hot weights: oh[:, 20 - c : 40 - c] is a
            # [128, 20] matrix whose only nonzero column (all ones) is c.
            oh = constp.tile([P, 2 * C], dt.bfloat16)
            nc.gpsimd.memset(oh[:], 0.0)
            nc.gpsimd.memset(oh[:, C : C + 1], 1.0)

            s_res = resp.tile([P, NITEMS], dt.float32)
            un_red = resp.tile([C, 4], dt.float32)
            red_scratch = resp.tile([C, 512], dt.float32)

            u_psum = [
                psump.tile([C, 512], dt.float32, name=f"u_psum{i}") for i in range(2)
            ]
            c_psum = [
                psump.tile([C, 512], dt.float32, name=f"c_psum{i}") for i in range(2)
            ]

            # Engine warmup during the DMA fill phase (HAM clock ramp).
            warm = constp.tile([P, 512], dt.bfloat16)
            nc.vector.memset(warm[:], 0.0)
            if WARM_MM:
                w_psum = psump.tile([C, 512], dt.float32)
                for i in range(WARM_MM):
                    nc.tensor.matmul(
                        w_psum[:], oh[:, 0:C], warm[:],
                        start=(i == 0), stop=(i == WARM_MM - 1),
                    )
            if WARM_VE:
                warm2 = constp.tile([P, 512], dt.bfloat16)
                warm3 = constp.tile([P, 512], dt.bfloat16)
                for i in range(WARM_VE):
                    nc.vector.tensor_scalar(warm2[:], warm[:], 0.0, None, Alu.max)
                    nc.scalar.activation(warm3[:], warm[:], Act.Exp)

            mm_started = {}

            def emit_mms(stream, bank, rhs, c, width, last=False):
                tgt = (u_psum if stream == 0 else c_psum)[bank]
                nj = width // 512
                for j in range(nj):
                    first = (stream, bank) not in mm_started
                    mm_started[(stream, bank)] = True
                    nc.tensor.matmul(
                        tgt[:],
                        oh[:, C - c : 2 * C - c],
                        rhs[:, j * 512 : (j + 1) * 512],
                        start=first,
                        stop=last and j == nj - 1,
                    )

            def fold(bank):
                """512->1 fold of a bank set's PSUM rows on the ACT engine."""
                nc.scalar.activation(
                    red_scratch[:], u_psum[bank][:], Act.Copy,
                    accum_out=un_red[:, bank : bank + 1],
                )
                nc.scalar.activation(
                    red_scratch[:], c_psum[bank][:], Act.Copy,
                    accum_out=un_red[:, 2 + bank : 2 + bank + 1],
                )

            pending = None  # (item idx, c, width, r_t, a_t, mk_t)

            def flush_pending(last=False):
                pidx, pc, pw, pr, pa, pmk = pending
                pbank = 0 if pidx <= BANK0_LAST else 1
                blast = last or pidx == BANK0_LAST
                w_t = datap.tile([P, pw], dt.bfloat16, tag="w")
                nc.vector.tensor_tensor(w_t[:], pa[:], pr[:], Alu.mult)
                emit_mms(1, pbank, pmk, pc, pw, last=blast)
                emit_mms(0, pbank, w_t, pc, pw, last=blast)
                if blast:
                    fold(pbank)

            for idx, (c, lo, width) in enumerate(ITEMS):
                x_t = datap.tile([P, width], dt.bfloat16, tag="x")
                # SWDGE DMA casts fp32 -> bf16 on the fly.
                nc.gpsimd.dma_start(x_t[:], x_dram[c, :, lo : lo + width])

                r_t = datap.tile([P, width], dt.bfloat16, tag="r")
                a_t = datap.tile([P, width], dt.bfloat16, tag="a")
                mk_t = datap.tile([P, width], dt.bfloat16, tag="mk")

                nc.vector.tensor_scalar(r_t[:], x_t[:], 0.0, None, Alu.max)
                nc.scalar.activation(
                    a_t[:], r_t[:], Act.Exp, accum_out=s_res[:, idx : idx + 1]
                )
                nc.vector.tensor_scalar(mk_t[:], x_t[:], 0.0, None, Alu.is_gt)

                if pending is not None:
                    flush_pending()
                pending = (idx, c, width, r_t, a_t, mk_t)

            flush_pending(last=True)
            nc.sync.dma_start(s_dram[:], s_res[:])
            nc.sync.dma_start(un_dram[:], un_red[:])

    nc.compile()
    return nc


def _get_program():
    if "nc" not in _CACHE:
        _CACHE["nc"] = _build_program()
    return _CACHE["nc"]


def _run(heatmap: np.ndarray, trace: bool = False):
    from concourse.bass_utils import run_bass_kernel_spmd

    nc = _get_program()
    in_maps = [
        {"x": np.ascontiguousarray(heatmap[i].reshape(C, P, F), dtype=np.float32)}
        for i in range(NCORES)
    ]
    return run_bass_kernel_spmd(nc, in_maps, list(range(NCORES)), trace=trace)


def _finalize(results) -> np.ndarray:
    """Host epilogue: a few hundred scalars per core -> entropy[n]."""
    out = np.zeros(N, dtype=np.float64)
    for n in range(NCORES):
        r = results[n]
        s_it = r["s_out"].astype(np.float64).sum(axis=0)        # [NITEMS]
        s_prime = np.zeros(C, dtype=np.float64)
        for idx, (c, _, _) in enumerate(ITEMS):
            s_prime[c] += s_it[idx]
        un = r["un_out"].astype(np.float64)                     # [C, 4]
        u = un[:, 0] + un[:, 1]
        cnt = un[:, 2] + un[:, 3]
        s = s_prime - (HW - cnt)                                # masked sum exp
        ent = np.zeros(C, dtype=np.float64)
        ok = s > 0
        ent[ok] = (np.log(s[ok]) - u[ok] / s[ok]) / LN2
        out[n] = ent.sum() / cnt.sum()
    return out.astype(np.float32)


def kernel(heatmap: np.ndarray) -> np.ndarray:
    heatmap = np.asarray(heatmap, dtype=np.float32)
    assert heatmap.shape == (N, C, H, W), heatmap.shape
    res = _run(heatmap, trace=False)
    return _finalize(res.results)
